# revision 1
# baseline (speedup 1.0000x reference)
"""Trainium2 Bass kernel: causal GQA self-attention (B=2, T=2048, C=1024,
16 q-heads / 4 kv-heads, rotary + q/k RMS-norm), sharded over 8 NeuronCores
as (batch x kv-group). Self-contained: kernel(**inputs) -> np.ndarray.
"""
import sys
from contextlib import ExitStack

for p in ("/opt/trn_rl_repo", "/root/.axon_site/_ro/trn_rl_repo"):
    if p not in sys.path:
        sys.path.insert(0, p)

import numpy as np
import ml_dtypes

import concourse.bass as bass
import concourse.mybir as mybir
from concourse.tile import TileContext
from concourse.masks import make_identity

F32 = mybir.dt.float32
BF16 = mybir.dt.bfloat16
NPBF16 = ml_dtypes.bfloat16

T, C, HQ, D = 2048, 1024, 4, 64
DQ = HQ * D
TC = T // 128
KC = C // 128
NJ = T // 512
EPS = 1.1920929e-7
NEG = -1e30


def _bcast_ap(sl, n, at=1):
    ap = list(sl.ap)
    ap.insert(at, [0, n])
    return bass.AP(tensor=sl.tensor, offset=sl.offset, ap=ap)


def _split_waits(nc, maxw=1):
    """Walrus in this toolchain allows 1 sem-wait per instruction; split extras
    onto preceding same-engine NoOps."""
    cnt = 0
    for f in nc.m.functions:
        for b in f.blocks:
            il = list(b.instructions)
            out = []
            changed = False
            for inst in il:
                si = inst.sync_info
                waits = list(si.on_wait) if si and si.on_wait else []
                if len(waits) > maxw:
                    chunks = [waits[i:i + maxw] for i in range(0, len(waits), maxw)]
                    for ch in chunks[:-1]:
                        cnt += 1
                        nop = mybir.InstNoOp(name=f"I-waitfix-{cnt}")
                        nop.engine = inst.engine
                        nop.sync_info = mybir.SyncInfo(on_wait=ch, on_update=[])
                        out.append(nop)
                    si.on_wait = chunks[-1]
                    inst.sync_info = si
                    changed = True
                out.append(inst)
            if changed:
                b.instructions = out
    return cnt


def _build_attn(ctx, tc, outs, ins):
    nc = tc.nc
    xT, wq, wkv, wo, cos2, sin2 = (
        ins["xT"], ins["wq"], ins["wkv"], ins["wo"], ins["cos2"], ins["sin2"])
    outT = outs["outT"]

    singles = ctx.enter_context(tc.tile_pool(name="singles", bufs=1))

    ident = singles.tile([128, 128], F32, tag="ident")
    make_identity(nc, ident)
    mask = singles.tile([128, 4, 512], F32, tag="mask")
    nc.gpsimd.memset(mask, 0.0)
    nc.gpsimd.affine_select(
        out=mask, in_=mask, compare_op=mybir.AluOpType.is_ge, fill=NEG,
        base=0, pattern=[[-128, 4], [1, 512]], channel_multiplier=-1)
    ones_row = singles.tile([1, 64], F32, tag="ones_row")
    nc.vector.memset(ones_row, 1.0)
    eps_t = singles.tile([128, 1], F32, tag="eps_t")
    nc.vector.memset(eps_t, EPS)

    xsb = singles.tile([128, KC, T], BF16, tag="xsb")
    nc.sync.dma_start(out=xsb, in_=xT.rearrange("(a p) t -> p a t", p=128))
    wq_sb = singles.tile([128, KC, DQ], BF16, tag="wq_sb")
    nc.sync.dma_start(out=wq_sb, in_=wq.rearrange("(a p) n -> p a n", p=128))
    wkv_sb = singles.tile([128, KC, 128], BF16, tag="wkv_sb")
    nc.sync.dma_start(out=wkv_sb, in_=wkv.rearrange("(a p) n -> p a n", p=128))
    wo_sb = singles.tile([128, 2, C], BF16, tag="wo_sb")
    nc.sync.dma_start(out=wo_sb, in_=wo.rearrange("(a p) o -> p a o", p=128))
    cos_sb = singles.tile([128, TC, 32], F32, tag="cos_sb")
    nc.sync.dma_start(out=cos_sb, in_=cos2.rearrange("(a p) d -> p a d", p=128))
    sin_sb = singles.tile([128, TC, 32], F32, tag="sin_sb")
    nc.sync.dma_start(out=sin_sb, in_=sin2.rearrange("(a p) d -> p a d", p=128))

    qt0 = singles.tile([128, T], BF16, tag="qt0")
    qt1 = singles.tile([128, T], BF16, tag="qt1")
    kt2 = singles.tile([128, T], BF16, tag="kt2")
    v_sb = singles.tile([128, TC, 65], BF16, tag="v_sb")
    nc.vector.memset(v_sb[:, :, 64:65], 1.0)
    yt0 = singles.tile([128, T], BF16, tag="yt0")
    yt1 = singles.tile([128, T], BF16, tag="yt1")
    qts = (qt0, qt1)
    yts = (yt0, yt1)

    # ---- Phase 1: projections + rope + rms + transposes ----
    with (
        tc.tile_pool(name="scratch", bufs=1) as scratch,
        tc.tile_pool(name="rtmp", bufs=2) as rtmp,
    ):
        q2 = scratch.tile([128, TC, DQ], F32, tag="q2")
        kn = scratch.tile([128, TC, 128], F32, tag="kn")

        with tc.tile_pool(name="pp", bufs=3, space="PSUM") as pp:
            for t_ in range(TC):
                qps = pp.tile([128, DQ], F32, tag="qps")
                for kc in range(KC):
                    nc.tensor.matmul(
                        qps, xsb[:, kc, t_ * 128:(t_ + 1) * 128], wq_sb[:, kc, :],
                        start=(kc == 0), stop=(kc == KC - 1))
                q3 = qps.rearrange("p (h d) -> p h d", h=HQ)
                x1, x2 = q3[:, :, 0:32], q3[:, :, 32:64]
                cb = _bcast_ap(cos_sb[:, t_, :], HQ)
                sb = _bcast_ap(sin_sb[:, t_, :], HQ)
                o3 = q2[:, t_, :].rearrange("p (h d) -> p h d", h=HQ)
                t1 = rtmp.tile([128, HQ, 32], F32, tag="t1")
                t2 = rtmp.tile([128, HQ, 32], F32, tag="t2")
                t3 = rtmp.tile([128, HQ, 32], F32, tag="t3")
                t4 = rtmp.tile([128, HQ, 32], F32, tag="t4")
                nc.vector.tensor_mul(t1, x1, cb)
                nc.vector.tensor_mul(t2, x2, sb)
                nc.vector.tensor_add(o3[:, :, 0:32], t1, t2)
                nc.vector.tensor_mul(t3, x1, sb)
                nc.vector.tensor_mul(t4, x2, cb)
                nc.vector.tensor_sub(o3[:, :, 32:64], t4, t3)

                kvps = pp.tile([128, 128], F32, tag="kvps")
                for kc in range(KC):
                    nc.tensor.matmul(
                        kvps, xsb[:, kc, t_ * 128:(t_ + 1) * 128], wkv_sb[:, kc, :],
                        start=(kc == 0), stop=(kc == KC - 1))
                nc.scalar.copy(v_sb[:, t_, 0:64], kvps[:, 64:128])
                kx1, kx2 = kvps[:, 0:32], kvps[:, 32:64]
                kc_ = cos_sb[:, t_, :]
                ks_ = sin_sb[:, t_, :]
                u1 = rtmp.tile([128, 32], F32, tag="u1")
                u2 = rtmp.tile([128, 32], F32, tag="u2")
                u3 = rtmp.tile([128, 32], F32, tag="u3")
                u4 = rtmp.tile([128, 32], F32, tag="u4")
                nc.vector.tensor_mul(u1, kx1, kc_)
                nc.vector.tensor_mul(u2, kx2, ks_)
                nc.vector.tensor_add(kn[:, t_, 0:32], u1, u2)
                nc.vector.tensor_mul(u3, kx1, ks_)
                nc.vector.tensor_mul(u4, kx2, kc_)
                nc.vector.tensor_sub(kn[:, t_, 32:64], u4, u3)

        with tc.tile_pool(name="sqpool", bufs=1) as sqpool:
            sq = sqpool.tile([128, TC, DQ], F32, tag="sq")
            nc.vector.tensor_mul(sq, q2, q2)
            mv = scratch.tile([128, TC, HQ], F32, tag="mv")
            nc.vector.tensor_reduce(
                mv, sq.rearrange("p t (h d) -> p t h d", d=D),
                axis=mybir.AxisListType.X, op=mybir.AluOpType.add)
            sd = scratch.tile([128, TC, HQ], F32, tag="sd")
            nc.scalar.activation(sd, mv, mybir.ActivationFunctionType.Sqrt,
                                 bias=eps_t, scale=1.0 / D)
            rsq = scratch.tile([128, TC, HQ], F32, tag="rsq")
            nc.vector.reciprocal(rsq, sd)
            nc.vector.tensor_mul(
                q2.rearrange("p t (h d) -> p t h d", d=D),
                q2.rearrange("p t (h d) -> p t h d", d=D),
                _bcast_ap(rsq, D, at=3))
            sqk = sqpool.tile([128, TC, 64], F32, tag="sqk")
            kn3 = kn.rearrange("p t (c d) -> p t c d", d=64)
            nc.vector.tensor_mul(sqk, kn3[:, :, 0, :], kn3[:, :, 0, :])
            mvk = scratch.tile([128, TC, 1], F32, tag="mvk")
            nc.vector.tensor_reduce(mvk, sqk, axis=mybir.AxisListType.X,
                                    op=mybir.AluOpType.add)
            sdk = scratch.tile([128, TC, 1], F32, tag="sdk")
            nc.scalar.activation(sdk, mvk, mybir.ActivationFunctionType.Sqrt,
                                 bias=eps_t, scale=1.0 / D)
            rsk = scratch.tile([128, TC, 1], F32, tag="rsk")
            nc.vector.reciprocal(rsk, sdk)
            rb = bass.AP(tensor=rsk.tensor, offset=rsk.offset,
                         ap=[rsk.ap[0], rsk.ap[1], [0, 64]])
            nc.vector.tensor_mul(kn3[:, :, 0, :], kn3[:, :, 0, :], rb)
            nc.vector.tensor_copy(kn3[:, :, 1, :], kn3[:, :, 0, :])

        with tc.tile_pool(name="tpp", bufs=3, space="PSUM") as tpp:
            for t_ in range(TC):
                for fs in range(2):
                    tps = tpp.tile([128, 128], F32, tag="tps")
                    nc.tensor.transpose(
                        tps, q2[:, t_, fs * 128:(fs + 1) * 128], ident)
                    cp = nc.scalar.copy if (t_ % 2) else nc.vector.tensor_copy
                    cp(qts[fs][:, t_ * 128:(t_ + 1) * 128], tps)
                tps2 = tpp.tile([128, 128], F32, tag="tps")
                nc.tensor.transpose(tps2, kn[:, t_, :], ident)
                cp = nc.vector.tensor_copy if (t_ % 2) else nc.scalar.copy
                cp(kt2[:, t_ * 128:(t_ + 1) * 128], tps2)

    # ---- Phase 2: attention ----
    with (
        tc.tile_pool(name="ptp", bufs=3) as ptp,
        tc.tile_pool(name="smallp", bufs=4) as smallp,
        tc.tile_pool(name="s4p", bufs=1, space="PSUM") as s4p,
        tc.tile_pool(name="o65p", bufs=2, space="PSUM") as o65p,
        tc.tile_pool(name="bcp", bufs=2, space="PSUM") as bcp,
    ):
        for j in range(NJ):
            for h in range(HQ):
                pair, base = h // 2, (h % 2) * 64
                tp = (base, 0) if base else None
                o65 = o65p.tile([65, 512], F32, tag="o65")
                for g in range(j + 1):
                    s4 = s4p.tile([128, 4, 512], F32, tag="s4")
                    for i in range(4):
                        c = 4 * g + i
                        nc.tensor.matmul(
                            s4[:, i, :],
                            kt2[base:base + 64, c * 128:(c + 1) * 128],
                            qts[pair][base:base + 64, j * 512:(j + 1) * 512],
                            start=True, stop=True, tile_position=tp)
                    if g == j:
                        nc.vector.tensor_add(s4, s4, mask)
                    pt = ptp.tile([128, 4, 512], BF16, tag="pt")
                    nc.scalar.activation(pt, s4, mybir.ActivationFunctionType.Exp,
                                         scale=0.125)
                    for i in range(4):
                        c = 4 * g + i
                        nc.tensor.matmul(
                            o65, v_sb[:, c, 0:65], pt[:, i, :],
                            start=(g == 0 and i == 0), stop=(g == j and i == 3))
                rec = smallp.tile([1, 512], F32, tag="rec")
                nc.vector.reciprocal(rec, o65[64:65, :])
                bc = bcp.tile([64, 512], F32, tag="bc")
                nc.tensor.matmul(bc, ones_row, rec, start=True, stop=True)
                bcs = smallp.tile([64, 512], F32, tag="bcs")
                nc.scalar.copy(bcs, bc)
                nc.vector.tensor_mul(
                    yts[pair][base:base + 64, j * 512:(j + 1) * 512],
                    o65[0:64, :], bcs)

    # ---- Phase 3: out projection ----
    with (
        tc.tile_pool(name="opp", bufs=3, space="PSUM") as opp,
        tc.tile_pool(name="osp", bufs=3) as osp,
    ):
        for m in range(8):
            for tq in range(NJ):
                ops_ = opp.tile([128, 512], F32, tag="ops")
                for fc in range(2):
                    nc.tensor.matmul(
                        ops_, wo_sb[:, fc, m * 128:(m + 1) * 128],
                        yts[fc][:, tq * 512:(tq + 1) * 512],
                        start=(fc == 0), stop=(fc == 1))
                ot = osp.tile([128, 512], F32, tag="ot")
                cp = nc.scalar.copy if ((m + tq) % 2) else nc.vector.tensor_copy
                cp(ot, ops_)
                nc.sync.dma_start(
                    out=outT[m * 128:(m + 1) * 128, tq * 512:(tq + 1) * 512],
                    in_=ot)


def _build_nc():
    nc = bass.Bass("TRN2", target_bir_lowering=False, debug=False, num_devices=8)
    ins = {
        "xT": nc.dram_tensor("xT", [1024, 2048], BF16, kind="ExternalInput").ap(),
        "wq": nc.dram_tensor("wq", [1024, 256], BF16, kind="ExternalInput").ap(),
        "wkv": nc.dram_tensor("wkv", [1024, 128], BF16, kind="ExternalInput").ap(),
        "wo": nc.dram_tensor("wo", [256, 1024], BF16, kind="ExternalInput").ap(),
        "cos2": nc.dram_tensor("cos2", [2048, 32], F32, kind="ExternalInput").ap(),
        "sin2": nc.dram_tensor("sin2", [2048, 32], F32, kind="ExternalInput").ap(),
    }
    outs = {"outT": nc.dram_tensor("outT", [1024, 2048], F32,
                                   kind="ExternalOutput").ap()}
    with TileContext(nc) as tc:
        with ExitStack() as ctx:
            _build_attn(ctx, tc, outs, ins)
    _split_waits(nc, maxw=1)
    return nc


def _shard_inputs(inputs, b, g):
    x, cos, sin = inputs["x"], inputs["cos"], inputs["sin"]
    Wq, Wk, Wv, Wo = inputs["Wq"], inputs["Wk"], inputs["Wv"], inputs["Wo"]
    qs, ks = slice(g * 256, (g + 1) * 256), slice(g * 64, (g + 1) * 64)
    return {
        "xT": np.ascontiguousarray(np.asarray(x[b]).T.astype(NPBF16)),
        "wq": np.ascontiguousarray(np.asarray(Wq[qs]).T.astype(NPBF16)),
        "wkv": np.ascontiguousarray(np.concatenate(
            [np.asarray(Wk[ks]).T, np.asarray(Wv[ks]).T], axis=1).astype(NPBF16)),
        "wo": np.ascontiguousarray(np.asarray(Wo[:, qs]).T.astype(NPBF16)),
        "cos2": np.ascontiguousarray(np.asarray(cos[0, :, 0, :]), dtype=np.float32),
        "sin2": np.ascontiguousarray(np.asarray(sin[0, :, 0, :]), dtype=np.float32),
    }


_STATE = None


def _get_state():
    global _STATE
    if _STATE is not None:
        return _STATE
    import jax
    from jax.sharding import Mesh, PartitionSpec, NamedSharding
    from jax.experimental.shard_map import shard_map
    from concourse.bass2jax import (
        _bass_exec_p, install_neuronx_cc_hook, partition_id_tensor)

    install_neuronx_cc_hook()
    nc = _build_nc()
    pname = nc.partition_id_tensor.name if nc.partition_id_tensor else None

    in_names, out_names, out_avals, zero_outs = [], [], [], []
    for alloc in nc.m.functions[0].allocations:
        if not isinstance(alloc, mybir.MemoryLocationSet):
            continue
        name = alloc.memorylocations[0].name
        if alloc.kind == "ExternalInput":
            if name != pname:
                in_names.append(name)
        elif alloc.kind == "ExternalOutput":
            out_names.append(name)
            shape = tuple(alloc.tensor_shape)
            dtype = mybir.dt.np(alloc.dtype)
            out_avals.append(jax.core.ShapedArray(shape, dtype))
            zero_outs.append(np.zeros(shape, dtype))
    n_params = len(in_names)
    all_names = in_names + out_names
    if pname is not None:
        all_names = all_names + [pname]

    def _body(*args):
        operands = list(args)
        if pname is not None:
            operands.append(partition_id_tensor())
        outs = _bass_exec_p.bind(
            *operands, out_avals=tuple(out_avals), in_names=tuple(all_names),
            out_names=tuple(out_names), lowering_input_output_aliases=(),
            sim_require_finite=True, sim_require_nnan=True, nc=nc)
        return tuple(outs)

    devices = jax.devices()[:8]
    mesh = Mesh(np.asarray(devices), ("core",))
    specs = (PartitionSpec("core"),) * (n_params + 1)
    sharded = jax.jit(shard_map(_body, mesh=mesh, in_specs=specs,
                                out_specs=(PartitionSpec("core"),),
                                check_rep=False))
    sharding = NamedSharding(mesh, PartitionSpec("core"))
    zeros = jax.device_put(
        np.zeros((8 * 1024, 2048), np.float32), sharding)
    _STATE = dict(sharded=sharded, sharding=sharding, in_names=in_names,
                  zeros=zeros, jax=jax)
    return _STATE


def _run_device(in_maps):
    st = _get_state()
    jax = st["jax"]
    concat_in = [np.concatenate([m[n] for m in in_maps], axis=0)
                 for n in st["in_names"]]
    dev_in = [jax.device_put(a, st["sharding"]) for a in concat_in]
    out = st["sharded"](*dev_in, st["zeros"])[0]
    return np.asarray(out).reshape(8, 1024, 2048)


def kernel(**inputs) -> np.ndarray:
    inputs = {k: np.asarray(v) for k, v in inputs.items()}
    in_maps = [_shard_inputs(inputs, b, g) for b in range(2) for g in range(4)]
    arr = _run_device(in_maps)
    out = np.zeros((2, 2048, 1024), np.float32)
    for c in range(8):
        out[c // 4] += arr[c].T
    return out



# revision 2
# speedup vs baseline: 1.0129x; 1.0129x over previous
"""Trainium2 Bass kernel: causal GQA self-attention (B=2, T=2048, C=1024,
16 q-heads / 4 kv-heads, rotary + q/k RMS-norm), sharded over 8 NeuronCores
as (batch x kv-group). Self-contained: kernel(**inputs) -> np.ndarray.

Single interleaved pipeline: fused QKV projection, grouped rope/RMS (norms
computed pre-rotation; rotation preserves them), software-pipelined causal
attention with column-trimmed scores/exp, and cross-group task interleaving
to keep the tensor engine ramped. Launch path uses fast-dispatch compile.
"""
import sys
from contextlib import ExitStack

for p in ("/opt/trn_rl_repo", "/root/.axon_site/_ro/trn_rl_repo"):
    if p not in sys.path:
        sys.path.insert(0, p)

import numpy as np
import ml_dtypes

import concourse.bass as bass
import concourse.mybir as mybir
from concourse.tile import TileContext
from concourse.masks import make_identity

F32 = mybir.dt.float32
BF16 = mybir.dt.bfloat16
NPBF16 = ml_dtypes.bfloat16

T, C, HQ, D = 2048, 1024, 4, 64
DQ = HQ * D          # 256 q dims per core
DKV = DQ + 2 * D     # 384 = q + k + v
TC = T // 128        # 16 t-chunks
KC = C // 128        # 8 contraction chunks
NJ = T // 512        # 4 query blocks / groups
EPS = 1.1920929e-7
EXP = mybir.ActivationFunctionType.Exp
SQRT = mybir.ActivationFunctionType.Sqrt


def _bcast_ap(sl, n, at=1):
    ap = list(sl.ap)
    ap.insert(at, [0, n])
    return bass.AP(tensor=sl.tensor, offset=sl.offset, ap=ap)


def _split_waits(nc, maxw=1):
    """Walrus in this toolchain allows 1 sem-wait per instruction; split extras
    onto preceding same-engine NoOps."""
    cnt = 0
    for f in nc.m.functions:
        for b in f.blocks:
            il = list(b.instructions)
            out = []
            changed = False
            for inst in il:
                si = inst.sync_info
                waits = list(si.on_wait) if si and si.on_wait else []
                if len(waits) > maxw:
                    chunks = [waits[i:i + maxw] for i in range(0, len(waits), maxw)]
                    for ch in chunks[:-1]:
                        cnt += 1
                        nop = mybir.InstNoOp(name=f"I-waitfix-{cnt}")
                        nop.engine = inst.engine
                        nop.sync_info = mybir.SyncInfo(on_wait=ch, on_update=[])
                        out.append(nop)
                    si.on_wait = chunks[-1]
                    inst.sync_info = si
                    changed = True
                out.append(inst)
            if changed:
                b.instructions = out
    return cnt


def _build_attn(ctx, tc, outs, ins):
    nc = tc.nc
    xT, wqkv, wo, cos2, sin2 = (
        ins["xT"], ins["wqkv"], ins["wo"], ins["cos2"], ins["sin2"])
    outT = outs["outT"]

    singles = ctx.enter_context(tc.tile_pool(name="singles", bufs=1))

    ident = singles.tile([128, 128], F32, tag="ident")
    make_identity(nc, ident)
    ones_row = singles.tile([1, 64], F32, tag="ones_row")
    nc.vector.memset(ones_row, 1.0)
    eps_t = singles.tile([128, 1], F32, tag="eps_t")
    nc.vector.memset(eps_t, EPS)

    wqkv_sb = singles.tile([128, KC, DKV], BF16, tag="wqkv_sb")
    nc.sync.dma_start(out=wqkv_sb, in_=wqkv.rearrange("(a p) n -> p a n", p=128))
    xsb = singles.tile([128, KC, T], BF16, tag="xsb")
    xr = xT.rearrange("(a p) t -> p a t", p=128)
    for c0 in range(4):
        nc.sync.dma_start(out=xsb[:, :, c0 * 128:(c0 + 1) * 128],
                          in_=xr[:, :, c0 * 128:(c0 + 1) * 128])
    cos_sb = singles.tile([128, TC, 32], F32, tag="cos_sb")
    nc.sync.dma_start(out=cos_sb, in_=cos2.rearrange("(a p) d -> p a d", p=128))
    sin_sb = singles.tile([128, TC, 32], F32, tag="sin_sb")
    nc.sync.dma_start(out=sin_sb, in_=sin2.rearrange("(a p) d -> p a d", p=128))
    nc.sync.dma_start(out=xsb[:, :, 512:1024], in_=xr[:, :, 512:1024])
    wo_sb = singles.tile([128, 2, C], BF16, tag="wo_sb")
    nc.sync.dma_start(out=wo_sb, in_=wo.rearrange("(a p) o -> p a o", p=128))
    for r in range(2, NJ):
        nc.sync.dma_start(out=xsb[:, :, r * 512:(r + 1) * 512],
                          in_=xr[:, :, r * 512:(r + 1) * 512])

    q2 = singles.tile([128, TC, DQ], F32, tag="q2")
    kn = singles.tile([128, TC, 128], F32, tag="kn")
    v_sb = singles.tile([128, TC, 65], BF16, tag="v_sb")
    nc.vector.memset(v_sb[:, :, 64:65], 1.0)
    qt0 = singles.tile([128, T], BF16, tag="qt0")
    qt1 = singles.tile([128, T], BF16, tag="qt1")
    kt2 = singles.tile([128, T], BF16, tag="kt2")
    yt0 = singles.tile([128, T], BF16, tag="yt0")
    yt1 = singles.tile([128, T], BF16, tag="yt1")
    qts = (qt0, qt1)
    yts = (yt0, yt1)

    mm = ctx.enter_context(tc.tile_pool(name="mm", bufs=2, space="PSUM"))
    s4p = ctx.enter_context(tc.tile_pool(name="s4p", bufs=4, space="PSUM"))
    o65p = ctx.enter_context(tc.tile_pool(name="o65p", bufs=2, space="PSUM"))
    stg = ctx.enter_context(tc.tile_pool(name="stg", bufs=2))
    rt = ctx.enter_context(tc.tile_pool(name="rt", bufs=2))
    ptp = ctx.enter_context(tc.tile_pool(name="ptp", bufs=3))
    smallp = ctx.enter_context(tc.tile_pool(name="smallp", bufs=4))
    osp = ctx.enter_context(tc.tile_pool(name="osp", bufs=3))

    stgts = [None] * NJ
    rts = [None] * NJ

    def qkv_chunk_tasks(g):
        def chunk(c, g=g):
            if c == 0:
                stgts[g] = stg.tile([128, 4, DKV], F32, tag="stg",
                                    name="stgt")
            stgt = stgts[g]
            t = g * 4 + c
            ps = mm.tile([128, 512], F32, tag="mm", name="ps")
            for kc in range(KC):
                nc.tensor.matmul(
                    ps[:, 0:DKV], xsb[:, kc, t * 128:(t + 1) * 128],
                    wqkv_sb[:, kc, :], start=(kc == 0), stop=(kc == KC - 1))
            nc.scalar.copy(stgt[:, c, :], ps[:, 0:DKV])
        return [lambda c=c: chunk(c) for c in range(4)]

    def qkv_group(g):
        for f in qkv_chunk_tasks(g):
            f()

    def rope_group_tasks(g):
        return [lambda: rope_rms(g), lambda: rope_q(g), lambda: rope_k(g)]

    def rope_group(g):
        for f in rope_group_tasks(g):
            f()

    def rope_rms(g):
        stgt = stgts[g]
        ts = slice(g * 4, g * 4 + 4)
        q3 = stgt[:, :, 0:DQ].rearrange("p c (h d) -> p c h d", h=HQ)
        k3 = stgt[:, :, DQ:DQ + 64]
        # rms scales from pre-rope values (rotation preserves the norm)
        sq = rt.tile([128, 4, DQ], F32, tag="sq")
        nc.vector.tensor_mul(sq, stgt[:, :, 0:DQ], stgt[:, :, 0:DQ])
        mv = rt.tile([128, 4, HQ], F32, tag="mv")
        nc.vector.tensor_reduce(
            mv, sq.rearrange("p c (h d) -> p c h d", d=D),
            axis=mybir.AxisListType.X, op=mybir.AluOpType.add)
        sd = rt.tile([128, 4, HQ], F32, tag="sd")
        nc.scalar.activation(sd, mv, SQRT, bias=eps_t, scale=1.0 / D)
        rsq = rt.tile([128, 4, HQ], F32, tag="rsq")
        nc.vector.reciprocal(rsq, sd)
        sk = rt.tile([128, 4, 64], F32, tag="sk")
        nc.gpsimd.tensor_mul(sk, k3, k3)
        mk = rt.tile([128, 4, 1], F32, tag="mk")
        nc.vector.tensor_reduce(mk, sk, axis=mybir.AxisListType.X,
                                op=mybir.AluOpType.add)
        sdk = rt.tile([128, 4, 1], F32, tag="sdk")
        nc.scalar.activation(sdk, mk, SQRT, bias=eps_t, scale=1.0 / D)
        rsk = rt.tile([128, 4, 1], F32, tag="rsk")
        nc.vector.reciprocal(rsk, sdk)
        # rms-scaled rotary tables
        cosr = rt.tile([128, 4, HQ, 32], F32, tag="cosr")
        nc.vector.tensor_mul(cosr, _bcast_ap(cos_sb[:, ts, :], HQ, at=2),
                             _bcast_ap(rsq, 32, at=3))
        sinr = rt.tile([128, 4, HQ, 32], F32, tag="sinr")
        nc.vector.tensor_mul(sinr, _bcast_ap(sin_sb[:, ts, :], HQ, at=2),
                             _bcast_ap(rsq, 32, at=3))
        rkb = bass.AP(tensor=rsk.tensor, offset=rsk.offset,
                      ap=[rsk.ap[0], rsk.ap[1], [0, 32]])
        cosk = rt.tile([128, 4, 32], F32, tag="cosk")
        nc.gpsimd.tensor_mul(cosk, cos_sb[:, ts, :], rkb)
        sink = rt.tile([128, 4, 32], F32, tag="sink")
        nc.gpsimd.tensor_mul(sink, sin_sb[:, ts, :], rkb)
        rts[g] = (cosr, sinr, cosk, sink)

    def rope_q(g):
        stgt = stgts[g]
        ts = slice(g * 4, g * 4 + 4)
        q3 = stgt[:, :, 0:DQ].rearrange("p c (h d) -> p c h d", h=HQ)
        cosr, sinr, cosk, sink = rts[g]
        # rope q -> q2
        x1, x2 = q3[:, :, :, 0:32], q3[:, :, :, 32:64]
        q2v = q2[:, ts, :].rearrange("p c (h d) -> p c h d", h=HQ)
        t1 = rt.tile([128, 4, HQ, 32], F32, tag="t1")
        t2 = rt.tile([128, 4, HQ, 32], F32, tag="t2")
        nc.vector.tensor_mul(t1, x1, cosr)
        nc.vector.tensor_mul(t2, x2, sinr)
        nc.vector.tensor_add(q2v[:, :, :, 0:32], t1, t2)
        nc.vector.tensor_mul(t1, x1, sinr)
        nc.vector.tensor_mul(t2, x2, cosr)
        nc.vector.tensor_sub(q2v[:, :, :, 32:64], t2, t1)
    def rope_k(g):
        stgt = stgts[g]
        ts = slice(g * 4, g * 4 + 4)
        k3 = stgt[:, :, DQ:DQ + 64]
        cosr, sinr, cosk, sink = rts[g]
        # rope k -> kn cols 0:64, duplicate to 64:128
        kx1, kx2 = k3[:, :, 0:32], k3[:, :, 32:64]
        u1 = rt.tile([128, 4, 32], F32, tag="u1")
        u2 = rt.tile([128, 4, 32], F32, tag="u2")
        nc.gpsimd.tensor_mul(u1, kx1, cosk)
        nc.gpsimd.tensor_mul(u2, kx2, sink)
        nc.gpsimd.tensor_add(kn[:, ts, 0:32], u1, u2)
        nc.gpsimd.tensor_mul(u1, kx1, sink)
        nc.gpsimd.tensor_mul(u2, kx2, cosk)
        nc.gpsimd.tensor_sub(kn[:, ts, 32:64], u2, u1)
        nc.gpsimd.tensor_copy(kn[:, ts, 64:128], kn[:, ts, 0:64])
        # v -> v_sb (bf16)
        for c in range(4):
            nc.gpsimd.tensor_copy(v_sb[:, g * 4 + c, 0:64],
                                  stgt[:, c, DQ + 64:DKV])

    def transp_group(g):
        for c in range(4):
            t = g * 4 + c
            tp = mm.tile([128, 512], F32, tag="mm")
            nc.tensor.transpose(tp[:, 0:128], q2[:, t, 0:128], ident)
            nc.tensor.transpose(tp[:, 128:256], q2[:, t, 128:256], ident)
            nc.tensor.transpose(tp[:, 256:384], kn[:, t, :], ident)
            sl = slice(t * 128, (t + 1) * 128)
            nc.scalar.copy(qt0[:, sl], tp[:, 0:128])
            nc.scalar.copy(qt1[:, sl], tp[:, 128:256])
            nc.vector.tensor_copy(kt2[:, sl], tp[:, 256:384])

    def attention_tasks(j):
        tasks = []
        jq = j * 512
        npair = 2 * (j + 1)
        for h in range(HQ):
            pair, base = h // 2, (h % 2) * 64
            tpos = (base, 0) if base else None
            st = {"pts": [None] * npair}

            def scores_pair(p, st=st, pair=pair, base=base, tpos=tpos):
                if p == 0:
                    st["o65"] = o65p.tile([65, 512], F32, tag="o65",
                                          name="o65")
                pt = ptp.tile([128, 2, 512], BF16, tag="pt", name="pt")
                st["pts"][p] = pt
                for i2 in range(2):
                    c = 2 * p + i2
                    i_loc = c - 4 * j
                    lo = i_loc * 128 if i_loc > 0 else 0
                    s4 = s4p.tile([128, 512], F32, tag="s4", name="s4")
                    nc.tensor.matmul(
                        s4[:, lo:512],
                        kt2[base:base + 64, c * 128:(c + 1) * 128],
                        qts[pair][base:base + 64, jq + lo:jq + 512],
                        start=True, stop=True, tile_position=tpos)
                    nc.scalar.activation(pt[:, i2, lo:512], s4[:, lo:512],
                                         EXP, scale=0.125)
                    if lo:
                        nc.gpsimd.memset(pt[:, i2, 0:lo], 0.0)
                    if i_loc >= 0:
                        nc.gpsimd.affine_select(
                            out=pt[:, i2, lo:lo + 128], in_=pt[:, i2, lo:lo + 128],
                            compare_op=mybir.AluOpType.is_ge, fill=0.0,
                            base=0, pattern=[[1, 128]], channel_multiplier=-1)

            def pv_pair(p, st=st):
                pt = st["pts"][p]
                for i2 in range(2):
                    c = 2 * p + i2
                    nc.tensor.matmul(
                        st["o65"], v_sb[:, c, :], pt[:, i2, :],
                        start=(c == 0), stop=(c == 4 * j + 3))

            def epilogue(st=st, pair=pair, base=base, h=h):
                o65 = st["o65"]
                rec = smallp.tile([1, 512], F32, tag="rec", name="rec")
                nc.vector.reciprocal(rec, o65[64:65, :])
                bc = mm.tile([128, 512], F32, tag="mm", name="bc")
                nc.tensor.matmul(bc[0:64, :], ones_row, rec,
                                 start=True, stop=True)
                bcs = smallp.tile([64, 512], F32, tag="bcs", name="bcs")
                cp = nc.scalar.copy if (h % 2) else nc.vector.tensor_copy
                cp(bcs, bc[0:64, :])
                nc.vector.tensor_mul(
                    yts[pair][base:base + 64, jq:jq + 512], o65[0:64, :], bcs)

            tasks.append(lambda p=0, f=scores_pair: f(p))
            for p in range(1, npair):
                tasks.append(lambda p=p, f=scores_pair, g=pv_pair:
                             (f(p), g(p - 1)))
            tasks.append(lambda f=pv_pair, p=npair - 1: f(p))
            tasks.append(epilogue)
        return tasks

    def outproj_tasks(j):
        def mtask(m, j=j):
            op = mm.tile([128, 512], F32, tag="mm", name="op")
            for fc in range(2):
                nc.tensor.matmul(
                    op, wo_sb[:, fc, m * 128:(m + 1) * 128],
                    yts[fc][:, j * 512:(j + 1) * 512],
                    start=(fc == 0), stop=(fc == 1))
            ot = osp.tile([128, 512], F32, tag="ot", name="ot")
            nc.vector.tensor_copy(ot, op)
            nc.sync.dma_start(
                out=outT[m * 128:(m + 1) * 128, j * 512:(j + 1) * 512],
                in_=ot)
        return [lambda m=m: mtask(m) for m in range(8)]

    def interleave(primary, extra):
        n, m = len(primary), len(extra)
        out, ei = [], 0
        for i, t in enumerate(primary):
            out.append(t)
            while ei < m and ei * n < m * (i + 1):
                out.append(extra[ei])
                ei += 1
        out.extend(extra[ei:])
        return out

    qkv_group(0)
    for task in interleave(qkv_chunk_tasks(1), rope_group_tasks(0)):
        task()
    for g in range(NJ):
        transp_group(g)
        extra = []
        if g < NJ - 1:
            extra += rope_group_tasks(g + 1)
        if g < NJ - 2:
            extra += qkv_chunk_tasks(g + 2)
        if g >= 1:
            extra += outproj_tasks(g - 1)
        for task in interleave(attention_tasks(g), extra):
            task()
    for task in outproj_tasks(NJ - 1):
        task()


def _build_nc():
    nc = bass.Bass("TRN2", target_bir_lowering=False, debug=False, num_devices=8)
    ins = {
        "xT": nc.dram_tensor("xT", [1024, 2048], BF16, kind="ExternalInput").ap(),
        "wqkv": nc.dram_tensor("wqkv", [1024, DKV], BF16, kind="ExternalInput").ap(),
        "wo": nc.dram_tensor("wo", [256, 1024], BF16, kind="ExternalInput").ap(),
        "cos2": nc.dram_tensor("cos2", [2048, 32], F32, kind="ExternalInput").ap(),
        "sin2": nc.dram_tensor("sin2", [2048, 32], F32, kind="ExternalInput").ap(),
    }
    outs = {"outT": nc.dram_tensor("outT", [1024, 2048], F32,
                                   kind="ExternalOutput").ap()}
    with TileContext(nc) as tc:
        with ExitStack() as ctx:
            _build_attn(ctx, tc, outs, ins)
    _split_waits(nc, maxw=1)
    return nc


def _shard_inputs(inputs, b, g):
    x, cos, sin = inputs["x"], inputs["cos"], inputs["sin"]
    Wq, Wk, Wv, Wo = inputs["Wq"], inputs["Wk"], inputs["Wv"], inputs["Wo"]
    qs, ks = slice(g * 256, (g + 1) * 256), slice(g * 64, (g + 1) * 64)
    return {
        "xT": np.ascontiguousarray(np.asarray(x[b]).T.astype(NPBF16)),
        "wqkv": np.ascontiguousarray(np.concatenate(
            [np.asarray(Wq[qs]).T, np.asarray(Wk[ks]).T, np.asarray(Wv[ks]).T],
            axis=1).astype(NPBF16)),
        "wo": np.ascontiguousarray(np.asarray(Wo[:, qs]).T.astype(NPBF16)),
        "cos2": np.ascontiguousarray(np.asarray(cos[0, :, 0, :]), dtype=np.float32),
        "sin2": np.ascontiguousarray(np.asarray(sin[0, :, 0, :]), dtype=np.float32),
    }


_STATE = None


def _get_state():
    global _STATE
    if _STATE is not None:
        return _STATE
    import jax
    from jax.sharding import Mesh, PartitionSpec, NamedSharding
    from jax.experimental.shard_map import shard_map
    from concourse.bass2jax import (
        _bass_exec_p, install_neuronx_cc_hook, partition_id_tensor)

    install_neuronx_cc_hook()
    nc = _build_nc()
    pname = nc.partition_id_tensor.name if nc.partition_id_tensor else None

    in_names, out_names, out_avals, zero_outs = [], [], [], []
    for alloc in nc.m.functions[0].allocations:
        if not isinstance(alloc, mybir.MemoryLocationSet):
            continue
        name = alloc.memorylocations[0].name
        if alloc.kind == "ExternalInput":
            if name != pname:
                in_names.append(name)
        elif alloc.kind == "ExternalOutput":
            out_names.append(name)
            shape = tuple(alloc.tensor_shape)
            dtype = mybir.dt.np(alloc.dtype)
            out_avals.append(jax.core.ShapedArray(shape, dtype))
            zero_outs.append(np.zeros(shape, dtype))
    n_params = len(in_names)
    all_names = in_names + out_names
    if pname is not None:
        all_names = all_names + [pname]

    def _body(*args):
        operands = list(args)
        if pname is not None:
            operands.append(partition_id_tensor())
        outs = _bass_exec_p.bind(
            *operands, out_avals=tuple(out_avals), in_names=tuple(all_names),
            out_names=tuple(out_names), lowering_input_output_aliases=(),
            sim_require_finite=True, sim_require_nnan=True, nc=nc)
        return tuple(outs)

    devices = jax.devices()[:8]
    mesh = Mesh(np.asarray(devices), ("core",))
    specs = (PartitionSpec("core"),) * (n_params + 1)
    fn = shard_map(_body, mesh=mesh, in_specs=specs,
                   out_specs=(PartitionSpec("core"),), check_rep=False)
    sharding = NamedSharding(mesh, PartitionSpec("core"))
    zeros = jax.device_put(
        np.zeros((8 * 1024, 2048), np.float32), sharding)
    dummies = []
    for n in in_names:
        for alloc in nc.m.functions[0].allocations:
            if (isinstance(alloc, mybir.MemoryLocationSet)
                    and alloc.memorylocations[0].name == n):
                shp = tuple(alloc.tensor_shape)
                dt = mybir.dt.np(alloc.dtype)
                dummies.append(jax.device_put(
                    np.zeros((8 * shp[0],) + shp[1:], dt), sharding))
                break
    try:
        from concourse.bass2jax import fast_dispatch_compile
        sharded = fast_dispatch_compile(
            lambda: jax.jit(fn).lower(*dummies, zeros).compile())
    except Exception:
        sharded = jax.jit(fn)
    _STATE = dict(sharded=sharded, sharding=sharding, in_names=in_names,
                  zeros=zeros, jax=jax)
    return _STATE


def _run_device(in_maps):
    st = _get_state()
    jax = st["jax"]
    concat_in = [np.concatenate([m[n] for m in in_maps], axis=0)
                 for n in st["in_names"]]
    dev_in = [jax.device_put(a, st["sharding"]) for a in concat_in]
    out = st["sharded"](*dev_in, st["zeros"])[0]
    return np.asarray(out).reshape(8, 1024, 2048)


def kernel(**inputs) -> np.ndarray:
    inputs = {k: np.asarray(v) for k, v in inputs.items()}
    in_maps = [_shard_inputs(inputs, b, g) for b in range(2) for g in range(4)]
    arr = _run_device(in_maps)
    out = np.zeros((2, 2048, 1024), np.float32)
    for c in range(8):
        out[c // 4] += arr[c].T
    return out


# revision 5
# speedup vs baseline: 1.0144x; 1.0014x over previous
"""Trainium2 Bass kernel: causal GQA self-attention (B=2, T=2048, C=1024,
16 q-heads / 4 kv-heads, rotary + q/k RMS-norm), sharded over 8 NeuronCores
as (batch x kv-group). Self-contained: kernel(**inputs) -> np.ndarray.

Single interleaved pipeline: fused QKV projection, grouped rope/RMS (norms
computed pre-rotation; rotation preserves them), software-pipelined causal
attention with column-trimmed scores/exp, and cross-group task interleaving
to keep the tensor engine ramped. Launch path uses fast-dispatch compile.
"""
import sys
from contextlib import ExitStack

for p in ("/opt/trn_rl_repo", "/root/.axon_site/_ro/trn_rl_repo"):
    if p not in sys.path:
        sys.path.insert(0, p)

import numpy as np
import ml_dtypes

import concourse.bass as bass
import concourse.mybir as mybir
from concourse.tile import TileContext
from concourse.masks import make_identity

F32 = mybir.dt.float32
BF16 = mybir.dt.bfloat16
NPBF16 = ml_dtypes.bfloat16

T, C, HQ, D = 2048, 1024, 4, 64
DQ = HQ * D          # 256 q dims per core
DKV = DQ + 2 * D     # 384 = q + k + v
TC = T // 128        # 16 t-chunks
KC = C // 128        # 8 contraction chunks
NJ = T // 512        # 4 query blocks / groups
EPS = 1.1920929e-7
EXP = mybir.ActivationFunctionType.Exp
SQRT = mybir.ActivationFunctionType.Sqrt


def _bcast_ap(sl, n, at=1):
    ap = list(sl.ap)
    ap.insert(at, [0, n])
    return bass.AP(tensor=sl.tensor, offset=sl.offset, ap=ap)


def _split_waits(nc, maxw=1):
    """Walrus in this toolchain allows 1 sem-wait per instruction; split extras
    onto preceding same-engine NoOps."""
    cnt = 0
    for f in nc.m.functions:
        for b in f.blocks:
            il = list(b.instructions)
            out = []
            changed = False
            for inst in il:
                si = inst.sync_info
                waits = list(si.on_wait) if si and si.on_wait else []
                if len(waits) > maxw:
                    chunks = [waits[i:i + maxw] for i in range(0, len(waits), maxw)]
                    for ch in chunks[:-1]:
                        cnt += 1
                        nop = mybir.InstNoOp(name=f"I-waitfix-{cnt}")
                        nop.engine = inst.engine
                        nop.sync_info = mybir.SyncInfo(on_wait=ch, on_update=[])
                        out.append(nop)
                    si.on_wait = chunks[-1]
                    inst.sync_info = si
                    changed = True
                out.append(inst)
            if changed:
                b.instructions = out
    return cnt


def _build_attn(ctx, tc, outs, ins):
    nc = tc.nc
    xT, wqkv, wo, cos2, sin2 = (
        ins["xT"], ins["wqkv"], ins["wo"], ins["cos2"], ins["sin2"])
    outT = outs["outT"]

    singles = ctx.enter_context(tc.tile_pool(name="singles", bufs=1))

    ident = singles.tile([128, 128], F32, tag="ident")
    make_identity(nc, ident)
    ones_row = singles.tile([1, 64], F32, tag="ones_row")
    nc.vector.memset(ones_row, 1.0)
    eps_t = singles.tile([128, 1], F32, tag="eps_t")
    nc.vector.memset(eps_t, EPS)

    wqkv_sb = singles.tile([128, KC, DKV], BF16, tag="wqkv_sb")
    wr = wqkv.rearrange("(a p) n -> p a n", p=128)
    xsb = singles.tile([128, KC, T], BF16, tag="xsb")
    xr = xT.rearrange("(a p) t -> p a t", p=128)
    nc.sync.dma_start(out=wqkv_sb[:, 0:2, :], in_=wr[:, 0:2, :])
    nc.sync.dma_start(out=xsb[:, 0:2, 0:128], in_=xr[:, 0:2, 0:128])
    for kc2 in range(2, KC, 2):
        nc.sync.dma_start(out=wqkv_sb[:, kc2:kc2 + 2, :],
                          in_=wr[:, kc2:kc2 + 2, :])
        nc.sync.dma_start(out=xsb[:, kc2:kc2 + 2, 0:128],
                          in_=xr[:, kc2:kc2 + 2, 0:128])
    nc.sync.dma_start(out=xsb[:, 0:2, 128:512], in_=xr[:, 0:2, 128:512])
    nc.sync.dma_start(out=xsb[:, 2:KC, 128:512], in_=xr[:, 2:KC, 128:512])
    cos_sb = singles.tile([128, TC, 32], F32, tag="cos_sb")
    nc.sync.dma_start(out=cos_sb, in_=cos2.rearrange("(a p) d -> p a d", p=128))
    sin_sb = singles.tile([128, TC, 32], F32, tag="sin_sb")
    nc.sync.dma_start(out=sin_sb, in_=sin2.rearrange("(a p) d -> p a d", p=128))
    nc.sync.dma_start(out=xsb[:, :, 512:1024], in_=xr[:, :, 512:1024])
    wo_sb = singles.tile([128, 2, C], BF16, tag="wo_sb")
    nc.sync.dma_start(out=wo_sb, in_=wo.rearrange("(a p) o -> p a o", p=128))
    for r in range(2, NJ):
        nc.sync.dma_start(out=xsb[:, :, r * 512:(r + 1) * 512],
                          in_=xr[:, :, r * 512:(r + 1) * 512])

    q2 = singles.tile([128, TC, DQ], F32, tag="q2")
    kn = singles.tile([128, TC, 128], F32, tag="kn")
    v_sb = singles.tile([128, TC, 65], BF16, tag="v_sb")
    nc.vector.memset(v_sb[:, :, 64:65], 1.0)
    qt0 = singles.tile([128, T], BF16, tag="qt0")
    qt1 = singles.tile([128, T], BF16, tag="qt1")
    kt2 = singles.tile([128, T], BF16, tag="kt2")
    yt0 = singles.tile([128, T], BF16, tag="yt0")
    yt1 = singles.tile([128, T], BF16, tag="yt1")
    qts = (qt0, qt1)
    yts = (yt0, yt1)

    mm = ctx.enter_context(tc.tile_pool(name="mm", bufs=2, space="PSUM"))
    s4p = ctx.enter_context(tc.tile_pool(name="s4p", bufs=4, space="PSUM"))
    o65p = ctx.enter_context(tc.tile_pool(name="o65p", bufs=2, space="PSUM"))
    stg = ctx.enter_context(tc.tile_pool(name="stg", bufs=2))
    rt = ctx.enter_context(tc.tile_pool(name="rt", bufs=2))
    ptp = ctx.enter_context(tc.tile_pool(name="ptp", bufs=3))
    smallp = ctx.enter_context(tc.tile_pool(name="smallp", bufs=4))
    osp = ctx.enter_context(tc.tile_pool(name="osp", bufs=3))

    stgts = [None] * NJ
    rts = [None] * NJ

    def qkv_chunk_tasks(g):
        def chunk(c, g=g):
            if c == 0:
                stgts[g] = stg.tile([128, 4, DKV], F32, tag="stg",
                                    name="stgt")
            stgt = stgts[g]
            t = g * 4 + c
            ps = mm.tile([128, 512], F32, tag="mm", name="ps")
            for kc in range(KC):
                nc.tensor.matmul(
                    ps[:, 0:DKV], xsb[:, kc, t * 128:(t + 1) * 128],
                    wqkv_sb[:, kc, :], start=(kc == 0), stop=(kc == KC - 1))
            nc.scalar.copy(stgt[:, c, 0:DQ + 64], ps[:, 0:DQ + 64])
            nc.vector.tensor_copy(v_sb[:, t, 0:64], ps[:, DQ + 64:DKV])
        return [lambda c=c: chunk(c) for c in range(4)]

    def qkv_group(g):
        for f in qkv_chunk_tasks(g):
            f()

    def rope_group_tasks(g):
        return [lambda: rope_rms(g), lambda: rope_q(g), lambda: rope_k(g)]

    def rope_group(g):
        for f in rope_group_tasks(g):
            f()

    def rope_rms(g):
        stgt = stgts[g]
        ts = slice(g * 4, g * 4 + 4)
        q3 = stgt[:, :, 0:DQ].rearrange("p c (h d) -> p c h d", h=HQ)
        k3 = stgt[:, :, DQ:DQ + 64]
        # rms scales from pre-rope values (rotation preserves the norm)
        sq = rt.tile([128, 4, DQ], F32, tag="sq")
        nc.vector.tensor_mul(sq, stgt[:, :, 0:DQ], stgt[:, :, 0:DQ])
        mv = rt.tile([128, 4, HQ], F32, tag="mv")
        nc.vector.tensor_reduce(
            mv, sq.rearrange("p c (h d) -> p c h d", d=D),
            axis=mybir.AxisListType.X, op=mybir.AluOpType.add)
        sd = rt.tile([128, 4, HQ], F32, tag="sd")
        nc.scalar.activation(sd, mv, SQRT, bias=eps_t, scale=1.0 / D)
        rsq = rt.tile([128, 4, HQ], F32, tag="rsq")
        nc.vector.reciprocal(rsq, sd)
        sk = rt.tile([128, 4, 64], F32, tag="sk")
        nc.gpsimd.tensor_mul(sk, k3, k3)
        mk = rt.tile([128, 4, 1], F32, tag="mk")
        nc.vector.tensor_reduce(mk, sk, axis=mybir.AxisListType.X,
                                op=mybir.AluOpType.add)
        sdk = rt.tile([128, 4, 1], F32, tag="sdk")
        nc.scalar.activation(sdk, mk, SQRT, bias=eps_t, scale=1.0 / D)
        rsk = rt.tile([128, 4, 1], F32, tag="rsk")
        nc.vector.reciprocal(rsk, sdk)
        # rms-scaled rotary tables
        cosr = rt.tile([128, 4, HQ, 32], F32, tag="cosr")
        nc.vector.tensor_mul(cosr, _bcast_ap(cos_sb[:, ts, :], HQ, at=2),
                             _bcast_ap(rsq, 32, at=3))
        sinr = rt.tile([128, 4, HQ, 32], F32, tag="sinr")
        nc.vector.tensor_mul(sinr, _bcast_ap(sin_sb[:, ts, :], HQ, at=2),
                             _bcast_ap(rsq, 32, at=3))
        rkb = bass.AP(tensor=rsk.tensor, offset=rsk.offset,
                      ap=[rsk.ap[0], rsk.ap[1], [0, 32]])
        cosk = rt.tile([128, 4, 32], F32, tag="cosk")
        nc.gpsimd.tensor_mul(cosk, cos_sb[:, ts, :], rkb)
        sink = rt.tile([128, 4, 32], F32, tag="sink")
        nc.gpsimd.tensor_mul(sink, sin_sb[:, ts, :], rkb)
        rts[g] = (cosr, sinr, cosk, sink)

    def rope_q(g):
        stgt = stgts[g]
        ts = slice(g * 4, g * 4 + 4)
        q3 = stgt[:, :, 0:DQ].rearrange("p c (h d) -> p c h d", h=HQ)
        cosr, sinr, cosk, sink = rts[g]
        # rope q -> q2
        x1, x2 = q3[:, :, :, 0:32], q3[:, :, :, 32:64]
        q2v = q2[:, ts, :].rearrange("p c (h d) -> p c h d", h=HQ)
        t1 = rt.tile([128, 4, HQ, 32], F32, tag="t1")
        t2 = rt.tile([128, 4, HQ, 32], F32, tag="t2")
        nc.vector.tensor_mul(t1, x1, cosr)
        nc.vector.tensor_mul(t2, x2, sinr)
        nc.vector.tensor_add(q2v[:, :, :, 0:32], t1, t2)
        nc.vector.tensor_mul(t1, x1, sinr)
        nc.vector.tensor_mul(t2, x2, cosr)
        nc.vector.tensor_sub(q2v[:, :, :, 32:64], t2, t1)
    def rope_k(g):
        stgt = stgts[g]
        ts = slice(g * 4, g * 4 + 4)
        k3 = stgt[:, :, DQ:DQ + 64]
        cosr, sinr, cosk, sink = rts[g]
        # rope k -> kn cols 0:64, duplicate to 64:128
        kx1, kx2 = k3[:, :, 0:32], k3[:, :, 32:64]
        u1 = rt.tile([128, 4, 32], F32, tag="u1")
        u2 = rt.tile([128, 4, 32], F32, tag="u2")
        nc.gpsimd.tensor_mul(u1, kx1, cosk)
        nc.gpsimd.tensor_mul(u2, kx2, sink)
        nc.gpsimd.tensor_add(kn[:, ts, 0:32], u1, u2)
        nc.gpsimd.tensor_mul(u1, kx1, sink)
        nc.gpsimd.tensor_mul(u2, kx2, cosk)
        nc.gpsimd.tensor_sub(kn[:, ts, 32:64], u2, u1)
        nc.gpsimd.tensor_copy(kn[:, ts, 64:128], kn[:, ts, 0:64])

    def transp_group(g):
        for c in range(4):
            t = g * 4 + c
            tp = mm.tile([128, 512], F32, tag="mm")
            nc.tensor.transpose(tp[:, 0:128], q2[:, t, 0:128], ident)
            nc.tensor.transpose(tp[:, 128:256], q2[:, t, 128:256], ident)
            nc.tensor.transpose(tp[:, 256:384], kn[:, t, :], ident)
            sl = slice(t * 128, (t + 1) * 128)
            nc.scalar.copy(qt0[:, sl], tp[:, 0:128])
            nc.scalar.copy(qt1[:, sl], tp[:, 128:256])
            nc.vector.tensor_copy(kt2[:, sl], tp[:, 256:384])

    def attention_tasks(j):
        tasks = []
        jq = j * 512
        npair = 2 * (j + 1)
        for h in range(HQ):
            pair, base = h // 2, (h % 2) * 64
            tpos = (base, 0) if base else None
            st = {"pts": [None] * npair}

            def scores_pair(p, st=st, pair=pair, base=base, tpos=tpos):
                if p == 0:
                    st["o65"] = o65p.tile([65, 512], F32, tag="o65",
                                          name="o65")
                pt = ptp.tile([128, 2, 512], BF16, tag="pt", name="pt")
                st["pts"][p] = pt
                for i2 in range(2):
                    c = 2 * p + i2
                    i_loc = c - 4 * j
                    lo = i_loc * 128 if i_loc > 0 else 0
                    s4 = s4p.tile([128, 512], F32, tag="s4", name="s4")
                    nc.tensor.matmul(
                        s4[:, lo:512],
                        kt2[base:base + 64, c * 128:(c + 1) * 128],
                        qts[pair][base:base + 64, jq + lo:jq + 512],
                        start=True, stop=True, tile_position=tpos)
                    nc.scalar.activation(pt[:, i2, lo:512], s4[:, lo:512],
                                         EXP, scale=0.125)
                    if lo:
                        nc.gpsimd.memset(pt[:, i2, 0:lo], 0.0)
                    if i_loc >= 0:
                        nc.gpsimd.affine_select(
                            out=pt[:, i2, lo:lo + 128], in_=pt[:, i2, lo:lo + 128],
                            compare_op=mybir.AluOpType.is_ge, fill=0.0,
                            base=0, pattern=[[1, 128]], channel_multiplier=-1)

            def pv_pair(p, st=st):
                pt = st["pts"][p]
                for i2 in range(2):
                    c = 2 * p + i2
                    nc.tensor.matmul(
                        st["o65"], v_sb[:, c, :], pt[:, i2, :],
                        start=(c == 0), stop=(c == 4 * j + 3))

            def epilogue(st=st, pair=pair, base=base, h=h):
                o65 = st["o65"]
                rec = smallp.tile([1, 512], F32, tag="rec", name="rec")
                nc.vector.reciprocal(rec, o65[64:65, :])
                bc = mm.tile([128, 512], F32, tag="mm", name="bc")
                nc.tensor.matmul(bc[0:64, :], ones_row, rec,
                                 start=True, stop=True)
                bcs = smallp.tile([64, 512], F32, tag="bcs", name="bcs")
                if j >= 2:
                    cp = nc.vector.tensor_copy
                else:
                    cp = nc.scalar.copy if (h % 2) else nc.vector.tensor_copy
                cp(bcs, bc[0:64, :])
                nc.vector.tensor_mul(
                    yts[pair][base:base + 64, jq:jq + 512], o65[0:64, :], bcs)

            tasks.append(lambda p=0, f=scores_pair: f(p))
            for p in range(1, npair):
                tasks.append(lambda p=p, f=scores_pair, g=pv_pair:
                             (f(p), g(p - 1)))
            tasks.append(lambda f=pv_pair, p=npair - 1: f(p))
            tasks.append(epilogue)
        return tasks

    def outproj_tasks(j):
        def mtask(m, j=j):
            op = mm.tile([128, 512], F32, tag="mm", name="op")
            for fc in range(2):
                nc.tensor.matmul(
                    op, wo_sb[:, fc, m * 128:(m + 1) * 128],
                    yts[fc][:, j * 512:(j + 1) * 512],
                    start=(fc == 0), stop=(fc == 1))
            ot = osp.tile([128, 512], BF16, tag="ot", name="ot")
            if j == NJ - 1:
                cp = nc.scalar.copy if (m % 2) else nc.vector.tensor_copy
            else:
                cp = nc.vector.tensor_copy
            cp(ot, op)
            nc.sync.dma_start(
                out=outT[m * 128:(m + 1) * 128, j * 512:(j + 1) * 512],
                in_=ot)
        return [lambda m=m: mtask(m) for m in range(8)]

    def interleave(primary, extra):
        n, m = len(primary), len(extra)
        out, ei = [], 0
        for i, t in enumerate(primary):
            out.append(t)
            while ei < m and ei * n < m * (i + 1):
                out.append(extra[ei])
                ei += 1
        out.extend(extra[ei:])
        return out

    qkv_group(0)
    for task in interleave(qkv_chunk_tasks(1), rope_group_tasks(0)):
        task()
    for g in range(NJ):
        transp_group(g)
        extra = []
        if g < NJ - 1:
            extra += rope_group_tasks(g + 1)
        if g < NJ - 2:
            extra += qkv_chunk_tasks(g + 2)
        if g >= 1:
            extra += outproj_tasks(g - 1)
        for task in interleave(attention_tasks(g), extra):
            task()
    for task in outproj_tasks(NJ - 1):
        task()


def _build_nc():
    nc = bass.Bass("TRN2", target_bir_lowering=False, debug=False, num_devices=8)
    ins = {
        "xT": nc.dram_tensor("xT", [1024, 2048], BF16, kind="ExternalInput").ap(),
        "wqkv": nc.dram_tensor("wqkv", [1024, DKV], BF16, kind="ExternalInput").ap(),
        "wo": nc.dram_tensor("wo", [256, 1024], BF16, kind="ExternalInput").ap(),
        "cos2": nc.dram_tensor("cos2", [2048, 32], F32, kind="ExternalInput").ap(),
        "sin2": nc.dram_tensor("sin2", [2048, 32], F32, kind="ExternalInput").ap(),
    }
    outs = {"outT": nc.dram_tensor("outT", [1024, 2048], BF16,
                                   kind="ExternalOutput").ap()}
    with TileContext(nc) as tc:
        with ExitStack() as ctx:
            _build_attn(ctx, tc, outs, ins)
    _split_waits(nc, maxw=1)
    return nc


def _shard_inputs(inputs, b, g):
    x, cos, sin = inputs["x"], inputs["cos"], inputs["sin"]
    Wq, Wk, Wv, Wo = inputs["Wq"], inputs["Wk"], inputs["Wv"], inputs["Wo"]
    qs, ks = slice(g * 256, (g + 1) * 256), slice(g * 64, (g + 1) * 64)
    return {
        "xT": np.ascontiguousarray(np.asarray(x[b]).T.astype(NPBF16)),
        "wqkv": np.ascontiguousarray(np.concatenate(
            [np.asarray(Wq[qs]).T, np.asarray(Wk[ks]).T, np.asarray(Wv[ks]).T],
            axis=1).astype(NPBF16)),
        "wo": np.ascontiguousarray(np.asarray(Wo[:, qs]).T.astype(NPBF16)),
        "cos2": np.ascontiguousarray(np.asarray(cos[0, :, 0, :]), dtype=np.float32),
        "sin2": np.ascontiguousarray(np.asarray(sin[0, :, 0, :]), dtype=np.float32),
    }


_STATE = None


def _get_state():
    global _STATE
    if _STATE is not None:
        return _STATE
    import jax
    from jax.sharding import Mesh, PartitionSpec, NamedSharding
    from jax.experimental.shard_map import shard_map
    from concourse.bass2jax import (
        _bass_exec_p, install_neuronx_cc_hook, partition_id_tensor)

    install_neuronx_cc_hook()
    nc = _build_nc()
    pname = nc.partition_id_tensor.name if nc.partition_id_tensor else None

    in_names, out_names, out_avals, zero_outs = [], [], [], []
    for alloc in nc.m.functions[0].allocations:
        if not isinstance(alloc, mybir.MemoryLocationSet):
            continue
        name = alloc.memorylocations[0].name
        if alloc.kind == "ExternalInput":
            if name != pname:
                in_names.append(name)
        elif alloc.kind == "ExternalOutput":
            out_names.append(name)
            shape = tuple(alloc.tensor_shape)
            dtype = mybir.dt.np(alloc.dtype)
            out_avals.append(jax.core.ShapedArray(shape, dtype))
            zero_outs.append(np.zeros(shape, dtype))
    n_params = len(in_names)
    all_names = in_names + out_names
    if pname is not None:
        all_names = all_names + [pname]

    def _body(*args):
        operands = list(args)
        if pname is not None:
            operands.append(partition_id_tensor())
        outs = _bass_exec_p.bind(
            *operands, out_avals=tuple(out_avals), in_names=tuple(all_names),
            out_names=tuple(out_names), lowering_input_output_aliases=(),
            sim_require_finite=True, sim_require_nnan=True, nc=nc)
        return tuple(outs)

    devices = jax.devices()[:8]
    mesh = Mesh(np.asarray(devices), ("core",))
    specs = (PartitionSpec("core"),) * (n_params + 1)
    fn = shard_map(_body, mesh=mesh, in_specs=specs,
                   out_specs=(PartitionSpec("core"),), check_rep=False)
    sharding = NamedSharding(mesh, PartitionSpec("core"))
    zeros = jax.device_put(
        np.zeros((8 * 1024, 2048), NPBF16), sharding)
    dummies = []
    for n in in_names:
        for alloc in nc.m.functions[0].allocations:
            if (isinstance(alloc, mybir.MemoryLocationSet)
                    and alloc.memorylocations[0].name == n):
                shp = tuple(alloc.tensor_shape)
                dt = mybir.dt.np(alloc.dtype)
                dummies.append(jax.device_put(
                    np.zeros((8 * shp[0],) + shp[1:], dt), sharding))
                break
    try:
        from concourse.bass2jax import fast_dispatch_compile
        sharded = fast_dispatch_compile(
            lambda: jax.jit(fn).lower(*dummies, zeros).compile())
    except Exception:
        sharded = jax.jit(fn)
    _STATE = dict(sharded=sharded, sharding=sharding, in_names=in_names,
                  zeros=zeros, jax=jax)
    return _STATE


def _run_device(in_maps):
    st = _get_state()
    jax = st["jax"]
    concat_in = [np.concatenate([m[n] for m in in_maps], axis=0)
                 for n in st["in_names"]]
    dev_in = [jax.device_put(a, st["sharding"]) for a in concat_in]
    out = st["sharded"](*dev_in, st["zeros"])[0]
    return np.asarray(out).reshape(8, 1024, 2048)


def kernel(**inputs) -> np.ndarray:
    inputs = {k: np.asarray(v) for k, v in inputs.items()}
    in_maps = [_shard_inputs(inputs, b, g) for b in range(2) for g in range(4)]
    arr = _run_device(in_maps)
    out = np.zeros((2, 2048, 1024), np.float32)
    for c in range(8):
        out[c // 4] += arr[c].T.astype(np.float32)
    return out
